# revision 27
# baseline (speedup 1.0000x reference)
"""Trainium2 Bass kernel for DeLanJacobianNet inverse dynamics.

Strategy (per core, pure data parallel over batch, 8 cores x 32768 samples):
  Everything is closed form (no autodiff):
    - hidden pre-activations z_i = w_i . q  (120 rows: jp0|jp1|g) via PE
    - sin(z_i + b_i) and cos = sin(z + b + pi/2) evaluated with ACT Sin.
      ACT Sin is only valid on [-pi, pi]; out-of-range args clamp to +-pi
      where sin == ~0.  So each (row, sin/cos) pair is expanded into one
      slot per 2pi-window covering its data range ("clamp-window" trick:
      sin(y) = sum_k sin(clamp(y - 2pi k)) exactly).  The ~329 slots are
      packed into 3 groups of <=128 partitions; per-slot window offsets
      ride the per-partition ACT bias AP; window duplicates simply reuse
      the same projection weight row, so the projection contraction sums
      the windows for free.
    - projection to 32 per-sample features via 3 accumulating PE matmuls
      with all mass/derivative scalings folded into weights host-side
      (ones row carries the biases)
    - proj outputs of 4 subchunks stacked on partition bands [128, 512],
      DVE 32x32 stream-transpose flips to sample-major
    - DVE elementwise combine evaluates the per-sample quadratic forms
      out = tau_m + c1 + c2 + g  (+ Ho @ qdd via per-partition scalars)
"""
import sys
import os

for _p in ("/opt/trn_rl_repo",):
    if _p not in sys.path:
        sys.path.insert(0, _p)

import numpy as np
from contextlib import ExitStack

import concourse.bass as bass
import concourse.tile as tile
from concourse import mybir
from concourse.bass_utils import run_bass_kernel_spmd

F32 = mybir.dt.float32
EPS = 1e-6
B = 262144
NCORES = 8
BC = B // NCORES            # 32768 samples per core
CHUNK = 1024                # samples per Z chunk (2 PSUM banks per group)
NSUB = 512                  # proj subchunk (1 PSUM bank)
NCHUNK = BC // CHUNK        # 32
NQUAD = BC // (4 * NSUB)    # 16 quads of 4 subchunks
NCB = NSUB // 32            # 16 col-blocks per subchunk
QROUND = 4                  # quads per combine round
NROUND = NQUAD // QROUND
NSLOT = (128, 128, 120)     # ACT slots per group (G3 rows 120..127 static)
TOTSLOT = sum(NSLOT)


def _folded_consts(inputs):
    f64 = lambda t: np.asarray(inputs[t], np.float64)
    m = f64("m")
    m0c, m1c = max(m[0], EPS), max(m[1], EPS)
    s0c, s1c = np.sqrt(m0c), np.sqrt(m1c)

    TRIL = np.tril_indices(3)
    Lm0 = np.zeros((3, 3)); Lm0[TRIL] = f64("L0")
    Lm1 = np.zeros((3, 3)); Lm1[TRIL] = f64("L1")
    Ho = (Lm0 @ Lm0.T)[:2, :2] + (Lm1 @ Lm1.T)[:2, :2]

    w = f64("jp0_W1")[:, 0]; b0 = f64("jp0_b1")
    W2a = f64("jp0_W2")[:2, :]; b2a = f64("jp0_b2")[:2]
    v = f64("jp1_W1"); b1v = f64("jp1_b1")
    W2u = f64("jp1_W2")[:4, :]; b2u = f64("jp1_b2")[:4]
    gW1 = f64("g_W1"); gb1 = f64("g_b1")
    gW2 = f64("g_W2"); gb2 = f64("g_b2")

    W1all = np.zeros((2, 120))
    W1all[0, 0:40] = w
    W1all[:, 40:80] = v.T
    W1all[:, 80:120] = gW1.T
    bias0 = np.concatenate([b0, b1v, gb1])    # [120]

    # per-(row, sin/cos) projection weight rows (feature layout: see combine)
    # 0:a~0 1:a~1 2:ap^0 3:ap^1 4..7:u~(k,j) 8..11:A 12..15:B 16..21:C 22:g0 23:g1
    Wsin = np.zeros((120, 32))
    Wcos = np.zeros((80, 32))
    Wsin[0:40, 0:2] = s0c * W2a.T
    Wcos[0:40, 2:4] = 4.0 * s0c * (W2a * w[None, :]).T
    Wsin[40:80, 4:8] = s1c * W2u.T
    Pw = s1c * (W2u * v[None, :, 0]).T
    Rw = s1c * (W2u * v[None, :, 1]).T
    for k in range(2):
        Wcos[40:80, 8 + 2 * k + 0] = 3.0 * Pw[:, 2 * k + 0]
        Wcos[40:80, 8 + 2 * k + 1] = 2.0 * Rw[:, 2 * k + 0] + Pw[:, 2 * k + 1]
        Wcos[40:80, 12 + 2 * k + 0] = 2.0 * Pw[:, 2 * k + 1] + Rw[:, 2 * k + 0]
        Wcos[40:80, 12 + 2 * k + 1] = 3.0 * Rw[:, 2 * k + 1]
        Wcos[40:80, 16 + 3 * k + 0] = Pw[:, 2 * k + 0]
        Wcos[40:80, 16 + 3 * k + 1] = Pw[:, 2 * k + 1] + Rw[:, 2 * k + 0]
        Wcos[40:80, 16 + 3 * k + 2] = Rw[:, 2 * k + 1]
    Wsin[80:120, 22:24] = gW2.T
    brow = np.zeros(32)
    brow[0:2] = s0c * b2a
    brow[4:8] = s1c * b2u
    brow[22:24] = gb2

    # ---- clamp-window slot assignment (depends on the actual data range)
    x = np.asarray(inputs["x"], np.float64)
    z = x[:, 0:2] @ W1all + bias0[None, :]
    lo, hi = z.min(0), z.max(0)
    tp = 2.0 * np.pi
    noclamp, clamp = [], []                    # (row, is_cos, k)
    for i in range(120):
        for k in range(int(round(lo[i] / tp)), int(round(hi[i] / tp)) + 1):
            off = -tp * k
            dst = noclamp if (lo[i] + off >= -np.pi and
                              hi[i] + off <= np.pi) else clamp
            dst.append((i, 0, k))
    for i in range(80):
        lc, hc = lo[i] + np.pi / 2, hi[i] + np.pi / 2
        for k in range(int(round(lc / tp)), int(round(hc / tp)) + 1):
            off = -tp * k
            dst = noclamp if (lc + off >= -np.pi and
                              hc + off <= np.pi) else clamp
            dst.append((i, 1, k))
    assert len(noclamp) <= 128, f"{len(noclamp)} no-clamp slots > 128"
    assert len(noclamp) + len(clamp) <= TOTSLOT, "too many window slots"
    kfar = int(round(hi[0] / tp)) + 3
    pad = (0, 2, kfar)                         # zero weight, garbage ok
    insts = (noclamp + [pad] * (128 - len(noclamp)) + clamp)
    insts += [pad] * (TOTSLOT - len(insts))

    w1g = np.zeros((3, 384))                   # row 0 = slot bias (ones row)
    wpg = np.zeros((384, 32))
    for idx, (i, pc, k) in enumerate(insts):
        g, j = divmod(idx, 128) if idx < 256 else (2, idx - 256)
        w1g[1:3, 128 * g + j] = W1all[:, i]
        w1g[0, 128 * g + j] = bias0[i] + (np.pi / 2 if pc == 1 else 0.0) - tp * k
        if pc == 0:
            wpg[128 * g + j] = Wsin[i]
        elif pc == 1:
            wpg[128 * g + j] = Wcos[i]
    wpg[256 + 120] = brow                     # ones row (G3 static row 120)

    aconst = np.zeros((8, CHUNK), np.float32)
    aconst[0, :] = 1.0
    hovals = np.tile(np.array([Ho[0, 0], Ho[0, 1], Ho[1, 0], Ho[1, 1]],
                              np.float32), (128, 1))
    return dict(
        w1g=np.ascontiguousarray(w1g, np.float32),
        wpg=np.ascontiguousarray(wpg, np.float32),
        aconst=aconst,
        hovals=np.ascontiguousarray(hovals, np.float32),
    )


def _core_in_map(xc, consts):
    xc = np.ascontiguousarray(xc, np.float32)
    xsm = np.ascontiguousarray(
        xc.reshape(NQUAD, 4, NCB, 32, 6).transpose(1, 3, 0, 2, 4)
        .reshape(128, NQUAD * NCB * 6))
    m = {"x": xc, "xsm": xsm}
    m.update(consts)
    return m


def _spill_waits(nc, limits=None, default=1):
    """walrus allows only one attached sync-wait per instruction; move the
    excess onto standalone EventSemaphore instructions on the same engine."""
    limits = limits or {}
    fn = nc.m.functions[0]
    wid = 0
    for bb in fn.blocks:
        out = []
        for inst in bb.instructions:
            si = inst.sync_info
            lim = limits.get(type(inst).__name__, default)
            if si is not None and len(si.on_wait) > lim:
                waits = list(si.on_wait)
                for w_ in waits[lim:]:
                    ev = mybir.InstEventSemaphore(
                        name=f"WSPILL-{wid}", ins=[], outs=[])
                    wid += 1
                    ev.engine = inst.engine
                    ev.sync_info = mybir.SyncInfo(on_wait=[w_], on_update=[])
                    out.append(ev)
                inst.sync_info = mybir.SyncInfo(
                    on_wait=waits[:lim], on_update=list(si.on_update))
            out.append(inst)
        bb.instructions = out
    return nc


def _build_nc():
    nc = bass.Bass()
    x_d = nc.declare_dram_parameter("x", [BC, 6], F32, isOutput=False)
    xsm_d = nc.declare_dram_parameter("xsm", [128, NQUAD * NCB * 6], F32,
                                      isOutput=False)
    w1_d = nc.declare_dram_parameter("w1g", [3, 384], F32, isOutput=False)
    wp_d = nc.declare_dram_parameter("wpg", [384, 32], F32, isOutput=False)
    ac_d = nc.declare_dram_parameter("aconst", [8, CHUNK], F32, isOutput=False)
    ho_d = nc.declare_dram_parameter("hovals", [128, 4], F32, isOutput=False)
    out_d = nc.declare_dram_parameter("out", [BC, 2], F32, isOutput=True)

    # sample index s = ((q*4 + r)*NCB + cb)*32 + i ; partition p = 32*r + i
    out_sm_view = out_d[:, :].rearrange(
        "(q r cb i) d -> r i q cb d", q=NQUAD, r=4, cb=NCB, i=32)
    xT_view = x_d[:, :].rearrange("n d -> d n")

    with tile.TileContext(nc) as tc, ExitStack() as ctx:
        consts = ctx.enter_context(tc.tile_pool(name="consts", bufs=1))
        persist = ctx.enter_context(tc.tile_pool(name="persist", bufs=1))
        zc_pool = ctx.enter_context(tc.tile_pool(name="zc", bufs=2))
        z_pool = ctx.enter_context(tc.tile_pool(name="z", bufs=1, space="PSUM"))
        a_pool = ctx.enter_context(tc.tile_pool(name="apool", bufs=2))
        p5_pool = ctx.enter_context(tc.tile_pool(name="p5", bufs=2,
                                                 space="PSUM"))
        tmp_pool = ctx.enter_context(tc.tile_pool(name="tmp", bufs=2))

        w1 = consts.tile([3, 384], F32, tag="w1")
        nc.sync.dma_start(w1[:, :], w1_d[:, :])
        zeros1 = consts.tile([128, 1], F32, tag="zeros1")
        nc.gpsimd.memset(zeros1[:, :], 0.0)
        wpdv = wp_d[:, :].rearrange("(g p) f -> g p f", g=3)
        wpv = []
        for g in range(3):
            wpt = consts.tile([128, 32], F32, tag=f"wp{g}", name=f"wp{g}")
            nc.sync.dma_start(wpt[:, :], wpdv[g])
            wpv.append(wpt[:, :])
        ho = consts.tile([128, 4], F32, tag="ho")
        nc.sync.dma_start(ho[:, :], ho_d[:, :])

        # x in sample-major layout matching the post-transpose partition map
        x_sm_flat = persist.tile([128, NQUAD * NCB * 6], F32, tag="xsm")
        nc.sync.dma_start(x_sm_flat[:, :], xsm_d[:, :])
        x_sm = x_sm_flat[:, :].rearrange("p (q cb d) -> p q cb d",
                                         q=NQUAD, cb=NCB, d=6)

        # transposed per-sample features, all quads
        pt = persist.tile([128, NQUAD * NSUB], F32, tag="pt")
        ptv = pt[:, :].rearrange("p (q cb f) -> p q cb f",
                                 q=NQUAD, cb=NCB, f=32)

        out_sm = persist.tile([128, NQUAD, NCB, 2], F32, tag="outsm")

        qt_tiles = []
        for qi in range(3):
            qtt = persist.tile([3, CHUNK], F32, tag=f"qtt{qi}")
            nc.gpsimd.memset(qtt[0:1, :], 1.0)
            qt_tiles.append(qtt)

        p5_tiles = {}

        def do_combine_round(rnd):
            q0 = rnd * QROUND
            qs = slice(q0, q0 + QROUND)
            cnt = [0]

            def T(n=1):
                cnt[0] += 1
                return tmp_pool.tile([128, QROUND, NCB, n], F32,
                                     tag=f"ctt{cnt[0]}",
                                     name=f"ct_{rnd}_{cnt[0]}")[:, :, :, :]

            P = lambda f0, n=1: ptv[:, qs, :, f0:f0 + n]
            X = lambda d0, n=1: x_sm[:, qs, :, d0:d0 + n]
            mul = lambda o, a_, b_: nc.vector.tensor_tensor(
                o, *bass.broadcast_tensor_aps(a_, b_), mybir.AluOpType.mult)
            add = lambda o, a_, b_: nc.vector.tensor_tensor(
                o, *bass.broadcast_tensor_aps(a_, b_), mybir.AluOpType.add)

            qd0, qd1 = X(2), X(3)
            qdd0, qdd1 = X(4), X(5)

            qq = T(3)                      # qd0^2, qd0*qd1, qd1^2
            mul(qq[:, :, :, 0:2], X(2, 2), qd0)
            mul(qq[:, :, :, 2:3], qd1, qd1)

            # s_k = u_k0*qd0 + u_k1*qd1 ; e_k = u_k0*qdd0 + u_k1*qdd1
            # u layout is (k,j): cols 4,5 = u00,u01 ; 6,7 = u10,u11
            se = T(4)                      # s0,s1,e0,e1
            t4 = T(4)
            mul(t4[:, :, :, 0:1], P(4), qd0)
            mul(t4[:, :, :, 1:2], P(6), qd0)
            mul(t4[:, :, :, 2:3], P(4), qdd0)
            mul(t4[:, :, :, 3:4], P(6), qdd0)
            t4b = T(4)
            mul(t4b[:, :, :, 0:1], P(5), qd1)
            mul(t4b[:, :, :, 1:2], P(7), qd1)
            mul(t4b[:, :, :, 2:3], P(5), qdd1)
            mul(t4b[:, :, :, 3:4], P(7), qdd1)
            add(se, t4, t4b)

            # sdot_k = C1_k*qq0 + C2_k*qq01 + C3_k*qq1  (C cols 16..21 k-major)
            sd = T(2)
            csl = ptv[:, qs, :, 16:22].rearrange(
                "p q c (k three) -> p q c k three", k=2, three=3)
            qqb = qq.unsqueeze(3).broadcast_to([128, QROUND, NCB, 2, 3])
            pr6 = tmp_pool.tile([128, QROUND, NCB, 2, 3], F32, tag="ctpr6",
                                name=f"ct6_{rnd}")[:, :, :, :, :]
            nc.vector.tensor_tensor(pr6, csl, qqb, mybir.AluOpType.mult)
            nc.vector.tensor_reduce(sd, pr6, mybir.AxisListType.X,
                                    mybir.AluOpType.add)

            # f_k = e_k + sdot_k
            fk = T(2)
            add(fk, se[:, :, :, 2:4], sd)

            # w_kj = A_kj*qd0 + B_kj*qd1   (A cols 8..11, B cols 12..15)
            wk = T(4)
            wkb = T(4)
            mul(wk, P(8, 4), qd0)
            mul(wkb, P(12, 4), qd1)
            add(wk, wk, wkb)

            # T1_j = sum_k u_kj * f_k ; T2_j = sum_k s_k * w_kj
            t1 = T(2)
            t2 = T(2)
            ujk = ptv[:, qs, :, 4:8].rearrange(
                "p q c (k j) -> p q c j k", k=2, j=2)
            fb = fk.unsqueeze(3).broadcast_to([128, QROUND, NCB, 2, 2])
            pr4 = tmp_pool.tile([128, QROUND, NCB, 2, 2], F32, tag="ctpr4",
                                name=f"ct4b_{rnd}")[:, :, :, :, :]
            nc.vector.tensor_tensor(pr4, ujk, fb, mybir.AluOpType.mult)
            nc.vector.tensor_reduce(t1, pr4, mybir.AxisListType.X,
                                    mybir.AluOpType.add)
            # w_kj lives in wk temp with (k,j) order -> view as j-major
            wv = wk.rearrange("p q c (k j) -> p q c j k", k=2, j=2)
            sb = se[:, :, :, 0:2].unsqueeze(3).broadcast_to(
                [128, QROUND, NCB, 2, 2])
            nc.vector.tensor_tensor(pr4, wv, sb, mybir.AluOpType.mult)
            nc.vector.tensor_reduce(t2, pr4, mybir.AxisListType.X,
                                    mybir.AluOpType.add)

            # alpha = a0^2 + a1^2 ; beta4 = a0*ap0 + a1*ap1
            ab = T(2)
            pr4b = T(4)
            mul(pr4b[:, :, :, 0:2], P(0, 2), P(0, 2))
            mul(pr4b[:, :, :, 2:4], P(0, 2), P(2, 2))
            av = pr4b.rearrange("p q c (two i) -> p q c two i", two=2, i=2)
            nc.vector.tensor_reduce(ab, av, mybir.AxisListType.X,
                                    mybir.AluOpType.add)

            # J0 = alpha*qdd0 + beta4*qq0
            j0 = T(1)
            j0b = T(1)
            mul(j0, ab[:, :, :, 0:1], qdd0)
            mul(j0b, ab[:, :, :, 1:2], qq[:, :, :, 0:1])
            add(j0, j0, j0b)

            # out = T1 + T2 + g (+J0 on col 0) + Ho @ qdd
            o2 = out_sm[:, qs, :, :]
            add(o2, t1, t2)
            add(o2, o2, P(22, 2))
            add(o2[:, :, :, 0:1], o2[:, :, :, 0:1], j0)
            for j in range(2):
                for l_ in range(2):
                    oj = o2[:, :, :, j:j + 1]
                    nc.vector.scalar_tensor_tensor(
                        oj, X(4 + l_), ho[:, 2 * j + l_:2 * j + l_ + 1], oj,
                        mybir.AluOpType.mult, mybir.AluOpType.add)

            for r in range(4):
                for q in range(q0, q0 + QROUND):
                    nc.sync.dma_start(out_sm_view[r][:, q, :, :],
                                      out_sm[32 * r:32 * r + 32, q, :, :])

        for ci in range(NCHUNK):
            c0 = ci * CHUNK
            qt = qt_tiles[ci % 3]
            nc.sync.dma_start(qt[1:3, :], xT_view[0:2, c0:c0 + CHUNK])

            ats = []
            for g in range(3):
                ng = NSLOT[g]
                zt = z_pool.tile([ng, CHUNK], F32, tag=f"z{g}",
                                 name=f"z{g}_{ci}")
                for s in range(CHUNK // NSUB):
                    nc.tensor.matmul(
                        zt[:, s * NSUB:(s + 1) * NSUB],
                        w1[:, 128 * g:128 * g + ng],
                        qt[:, s * NSUB:(s + 1) * NSUB], start=True, stop=True)
                at = a_pool.tile([128, CHUNK], F32, tag=f"a{g}",
                                 name=f"a{g}_{ci}")
                if g == 2:
                    nc.sync.dma_start(at[120:128, :], ac_d[0:8, :])
                if g == 0:
                    src_ap = zt[:, :]
                else:
                    zc = zc_pool.tile([ng, CHUNK], F32, tag=f"zc{g}",
                                      name=f"zc{g}_{ci}")
                    nc.vector.tensor_scalar(
                        zc[:, :], zt[:, :], float(-np.pi), float(np.pi),
                        mybir.AluOpType.max, mybir.AluOpType.min)
                    src_ap = zc[:, :]
                nc.scalar.activation(at[0:ng, :], src_ap,
                                     mybir.ActivationFunctionType.Sin,
                                     bias=zeros1[0:ng, 0:1])
                ats.append(at)

            for s in range(CHUNK // NSUB):
                sc = ci * (CHUNK // NSUB) + s     # global subchunk
                q, r = sc // 4, sc % 4
                if r == 0:
                    p5_tiles[q] = p5_pool.tile([128, NSUB], F32, tag="p5",
                                               name=f"p5_{q}")
                p5 = p5_tiles[q]
                sl = slice(s * NSUB, (s + 1) * NSUB)
                tp_ = (0, 32 * r)
                for g in range(3):
                    nc.tensor.matmul(p5[32 * r:32 * r + 32, :], wpv[g],
                                     ats[g][0:128, sl],
                                     start=(g == 0), stop=(g == 2),
                                     tile_position=tp_)
                if r == 3:
                    nc.vector.transpose(
                        pt[:, q * NSUB:(q + 1) * NSUB], p5[:, :])
                    del p5_tiles[q]
                    if (q + 1) % QROUND == 0:
                        do_combine_round(q // QROUND)

    _spill_waits(nc)
    return nc


_CACHED = {}


def kernel(**inputs):
    inputs = {k: np.asarray(v) for k, v in inputs.items()}
    x = np.ascontiguousarray(inputs["x"], np.float32)
    assert x.shape == (B, 6)
    consts = _folded_consts(inputs)

    if "nc" not in _CACHED:
        _CACHED["nc"] = _build_nc()
    nc = _CACHED["nc"]

    in_maps = [_core_in_map(x[c * BC:(c + 1) * BC], consts)
               for c in range(NCORES)]

    res = run_bass_kernel_spmd(nc, in_maps, list(range(NCORES)))
    out = np.concatenate([res.results[c]["out"] for c in range(NCORES)],
                         axis=0)
    return out.astype(np.float32)
